# revision 4
# baseline (speedup 1.0000x reference)
"""Event-RGB dynamic fusion module on 8 trn2 NeuronCores (v2).

Per-pixel dynamic 3x3 depthwise kernels predicted from concat(rgb, event)
via two 1x1 convs + relu, applied to reflect-padded rgb.

Sharding: 8 shards = (batch b in 0..3) x (H half in {0,1}); each core gets
reflect-padded rgb slabs (two bf16 copies at element offsets 0/1 so every
3x3-shift view stays 4-byte aligned for DVE 2x mode), a bf16 event slab,
and replicated pre-laid-out bf16 weights. Fully data-parallel, no
collectives.

v2 changes vs v1:
- mm1/mm2 merge the two 64-row pixel halves into single matmuls via
  block-structured weights (A-half at psum partitions 0-63, B at 64-127
  for mm2; interleaved 32-col groups for mm1), halving PE stream time.
- mm2 lhsT is block-diagonal [K=64 (hA|hB), M=128 (cA|cB)], one tap per
  matmul pair (2x N=512), dk tile [128 = cA|cB, 1024 px].
- apply: 8 of 9 taps are copied PSUM->SBUF bf16 (bias fused) by ACT (7)
  and DVE (1), pair-multiplied against 4D overlapping patch views as
  2x-mode TTs on DVE/GPSIMD; center tap S stays a direct 1x STT from
  PSUM. Add tree operates on [128,2048] pair tiles.
- output is written bf16 and widened to fp32 on the host.
"""

import os
from contextlib import ExitStack

import ml_dtypes
import numpy as np

import bass_rust
import concourse.bass as bass
import concourse.bacc as bacc
import concourse.mybir as mybir
import concourse.tile as tile
from concourse.bass_utils import run_bass_kernel_spmd

B, C, H, W = 4, 64, 256, 256
CEV, KK, MID = 32, 3, 32
NCORES = 8
SHARD_H = 128          # rows per core
HALF = 64              # rows per half (partition-packing of pixel halves)
RBLK = 16              # rows per half per block
NBLK = HALF // RBLK    # 4
WE = 260               # padded row length (even, so shifted views stay aligned)
SUBR = 4               # rows per half per sub-slice (=1024 px per half)
NSUB = RBLK // SUBR    # 4
F32 = mybir.dt.float32
BF16 = mybir.dt.bfloat16
AOP = mybir.AluOpType
RELU = mybir.ActivationFunctionType.Relu
IDENT = mybir.ActivationFunctionType.Identity
BF = ml_dtypes.bfloat16

# tap index ij = 3*(di+1) + (dj+1)
S_TAP = 4                       # (0,0) center: direct STT from PSUM on DVE
# pairs: (name, [ij_a, ij_b], slab, tap_stride_elems, base_row_off, base_col)
# slab "e" = even (dj=0, col base 2), "o" = odd (dj=-1 -> 0, dj=+1 -> 2)
PAIRS = [
    ("E",  [1, 7], "e", 2 * WE, -1, 2),   # (-1,0) & (+1,0): row-stride pair
    ("O1", [0, 2], "o", 2,      -1, 0),   # (-1,-1) & (-1,+1): col-stride pair
    ("O2", [3, 5], "o", 2,       0, 0),   # (0,-1) & (0,+1)
    ("O3", [6, 8], "o", 2,      +1, 0),   # (+1,-1) & (+1,+1)
]
# engine for each pair product: "D"=DVE tensor_tensor, "G"=GPSIMD
PROD_ENG = {"E": "D", "O1": "D", "O2": "G", "O3": "D"}
# engine for each of the 8 tap copies (pair name, half): "A"=ACT, "V"=DVE
COPY_ENG = {("E", 0): "A", ("E", 1): "A", ("O1", 0): "A", ("O1", 1): "A",
            ("O2", 0): "A", ("O2", 1): "A", ("O3", 0): "A", ("O3", 1): "A"}
# engines for adds: T=E+O1, U=O2+O3, V=T+U (2048 each), Wf=V0+V1, out=Wf+S
ADD_ENG = {"T": "D", "U": "G", "V": "D", "Wf": "D", "out": "D"}

_cache = {}


def _pair_view(flat_ap, pstride, tap_stride, base):
    """Overlapping [128, 2, 4, 256] view of a flat [128, N] slab tile."""
    v = flat_ap.copy()
    v.ap = bass_rust.VecI64Pair(
        [[pstride, 128], [tap_stride, 2], [WE, SUBR], [1, W]])
    v.offset = flat_ap.offset + base
    return v


def _build():
    nc = bacc.Bacc("TRN2", target_bir_lowering=False, debug=False)
    rgbe = nc.dram_tensor("rgbe", [C, SHARD_H + 2, WE], BF16, kind="ExternalInput").ap()
    rgbo = nc.dram_tensor("rgbo", [C, SHARD_H + 2, WE], BF16, kind="ExternalInput").ap()
    ev = nc.dram_tensor("ev", [CEV, SHARD_H, W], BF16, kind="ExternalInput").ap()
    w1 = nc.dram_tensor("w1", [128, 384], BF16, kind="ExternalInput").ap()
    w2 = nc.dram_tensor("w2", [128, 9 * 128], BF16, kind="ExternalInput").ap()
    bi = nc.dram_tensor("bi", [128, 10], F32, kind="ExternalInput").ap()
    out = nc.dram_tensor("out", [C, SHARD_H, W], BF16, kind="ExternalOutput").ap()

    with tile.TileContext(nc) as tc, ExitStack() as ctx:
        _kernel(ctx, tc, rgbe, rgbo, ev, w1, w2, bi, out)
    nc.compile()
    return nc


def _kernel(ctx, tc, rgbe, rgbo, ev, w1, w2, bi, out):
    nc = tc.nc
    consts = ctx.enter_context(tc.tile_pool(name="consts", bufs=1))
    rgb_p = ctx.enter_context(tc.tile_pool(name="rgb", bufs=2))
    ev_p = ctx.enter_context(tc.tile_pool(name="evp", bufs=2))
    h4_p = ctx.enter_context(tc.tile_pool(name="h4", bufs=2))
    dkb_p = ctx.enter_context(tc.tile_pool(name="dkb", bufs=5))
    sprod_p = ctx.enter_context(tc.tile_pool(name="sprod", bufs=2))
    prod_p = ctx.enter_context(tc.tile_pool(name="prod", bufs=6))
    acc_p = ctx.enter_context(tc.tile_pool(name="acc", bufs=4))
    outt_p = ctx.enter_context(tc.tile_pool(name="outt", bufs=4))
    ph_p = ctx.enter_context(tc.tile_pool(name="psum_h", bufs=1, space="PSUM"))
    pdk_p = ctx.enter_context(tc.tile_pool(name="psum_dk", bufs=3, space="PSUM"))

    w1t = consts.tile([128, 384], BF16)
    nc.sync.dma_start(w1t[:], w1[:])
    w2t = consts.tile([128, 9 * 128], BF16)
    nc.sync.dma_start(w2t[:], w2[:])
    bt = consts.tile([128, 10], F32)
    nc.sync.dma_start(bt[:], bi[:])

    for t in range(NBLK):
        rge = rgb_p.tile([128, (RBLK + 2) * WE], BF16, tag="rge")
        nc.sync.dma_start(rge[0:64, :], rgbe[:, t * RBLK:t * RBLK + RBLK + 2, :])
        nc.sync.dma_start(
            rge[64:128, :], rgbe[:, HALF + t * RBLK:HALF + t * RBLK + RBLK + 2, :])
        rgo = rgb_p.tile([128, (RBLK + 2) * WE], BF16, tag="rgo")
        nc.sync.dma_start(rgo[0:64, :], rgbo[:, t * RBLK:t * RBLK + RBLK + 2, :])
        nc.sync.dma_start(
            rgo[64:128, :], rgbo[:, HALF + t * RBLK:HALF + t * RBLK + RBLK + 2, :])
        evt = ev_p.tile([128, RBLK * W], BF16)
        nc.sync.dma_start(evt[64:96, :], ev[:, t * RBLK:t * RBLK + RBLK, :])
        nc.sync.dma_start(
            evt[96:128, :], ev[:, HALF + t * RBLK:HALF + t * RBLK + RBLK, :])

        rgev = rge[:].rearrange("p (r w) -> p r w", w=WE)      # [128, 18, 260]
        rgov = rgo[:].rearrange("p (r w) -> p r w", w=WE)
        evv = evt[:].rearrange("p (r w) -> p r w", w=W)        # [128, 16, 256]
        pstride_e = rge[:].ap[0][0]
        pstride_o = rgo[:].ap[0][0]

        for s in range(NSUB):
            r0 = SUBR * s
            # ---- mm1: h4[128, 1024] = relu(b1 + W1 @ concat(rgb, ev)),
            # A-half at psum partitions {0-31, 64-95}, B at {32-63, 96-127}
            ph = ph_p.tile([128, 2 * 512], F32)
            for nh in range(2):
                pr = r0 + 2 * nh
                dst = ph[:, 512 * nh:512 * nh + 512]
                # NB: two K=64 matmuls at (0,0)+(64,0) in ONE accumulation
                # group hang the PE on hw; split into two baseline-style
                # K64+K32 groups, second group opening with start=False so
                # the PSUM bank keeps accumulating.
                nc.tensor.matmul(dst, w1t[0:64, 0:128],
                                 rgev[0:64, pr + 1:pr + 3, 2:258],
                                 start=True, stop=False, tile_position=(0, 0))
                nc.tensor.matmul(dst, w1t[64:96, 256:384],
                                 evv[64:96, pr:pr + 2, :],
                                 start=False, stop=True, tile_position=(64, 0))
                nc.tensor.matmul(dst, w1t[64:128, 128:256],
                                 rgev[64:128, pr + 1:pr + 3, 2:258],
                                 start=False, stop=False, tile_position=(64, 0))
                nc.tensor.matmul(dst, w1t[96:128, 0:128],
                                 evv[96:128, pr:pr + 2, :],
                                 start=False, stop=True, tile_position=(96, 0))
            h4 = h4_p.tile([128, 1024], BF16)
            nc.scalar.activation(h4[:], ph[:], RELU, bias=bt[:, 0:1], scale=1.0)

            def mm2(ij, cp):
                dk = pdk_p.tile([128, 1024], F32, name=f"dk{ij}", tag="dk")
                for nh in range(2):
                    nc.tensor.matmul(
                        dk[:, 512 * nh:512 * nh + 512],
                        w2t[64 * cp:64 * cp + 64, 128 * ij:128 * ij + 128],
                        h4[64 * cp:64 * cp + 64, 512 * nh:512 * nh + 512],
                        start=True, stop=True, tile_position=(64 * cp, 0))
                return dk

            # ---- mm2 + apply: 4 pairs + center tap ----
            prods = {}
            cp = 0
            for name, ijs, slab, tstride, drow, bcol in PAIRS:
                dkb = dkb_p.tile([128, 2048], BF16)
                for hfi, ij in enumerate(ijs):
                    dk = mm2(ij, cp)
                    cp ^= 1
                    half = dkb[:, 1024 * hfi:1024 * hfi + 1024]
                    if COPY_ENG[(name, hfi)] == "A":
                        nc.scalar.activation(half, dk[:], IDENT,
                                             bias=bt[:, 1 + ij:2 + ij], scale=1.0)
                    else:
                        nc.vector.tensor_scalar_add(half, dk[:],
                                                    bt[:, 1 + ij:2 + ij])
                flat = rge[:] if slab == "e" else rgo[:]
                pstr = pstride_e if slab == "e" else pstride_o
                base = (r0 + 1 + drow) * WE + bcol
                patch = _pair_view(flat, pstr, tstride, base)
                prod = prod_p.tile([128, 2048], BF16)
                dkbv = dkb[:].rearrange("p (t r w) -> p t r w", t=2, w=W)
                prodv = prod[:].rearrange("p (t r w) -> p t r w", t=2, w=W)
                eng = nc.gpsimd if PROD_ENG[name] == "G" else nc.vector
                eng.tensor_tensor(prodv[:], dkbv[:], patch, op=AOP.mult)
                prods[name] = prod

            # center tap: direct STT from PSUM
            dkS = mm2(S_TAP, cp)
            sp = sprod_p.tile([128, 1024], BF16)
            spv = sp[:].rearrange("p (r w) -> p r w", w=W)
            patchS = rgev[:, r0 + 1:r0 + 5, 2:258]
            nc.vector.scalar_tensor_tensor(spv[:], dkS[:],
                                           bt[:, 1 + S_TAP:2 + S_TAP],
                                           patchS[:], op0=AOP.add, op1=AOP.mult)

            def tadd(key, shape, a, b, pool=acc_p, dt=BF16):
                r = pool.tile(shape, dt, tag="acc", name=f"acc{key}")
                eng = nc.gpsimd if ADD_ENG[key] == "G" else nc.vector
                eng.tensor_tensor(r[:], a, b, op=AOP.add)
                return r

            tT = tadd("T", [128, 2048], prods["E"][:], prods["O1"][:])
            tU = tadd("U", [128, 2048], prods["O2"][:], prods["O3"][:])
            tV = tadd("V", [128, 2048], tT[:], tU[:])
            tW = tadd("Wf", [128, 1024], tV[:, 0:1024], tV[:, 1024:2048])
            ot = tadd("out", [128, 1024], tW[:], sp[:], pool=outt_p)

            otv = ot[:].rearrange("p (r w) -> p r w", w=W)
            ra = t * RBLK + r0
            nc.sync.dma_start(out[:, ra:ra + SUBR, :], otv[0:64, :, :])
            nc.sync.dma_start(out[:, HALF + ra:HALF + ra + SUBR, :],
                              otv[64:128, :, :])


def _prep_consts(W1, b1, W2, b2):
    # mm1 lhsT blocks: psum col m -> {0-31: mA, 32-63: mB, 64-95: mA, 96-127: mB}
    w1sb = np.zeros((128, 384), np.float32)
    for m in range(32):
        for rep in (0, 64):
            w1sb[0:64, m + rep] = W1[m, 0:64]                 # rgbA
            w1sb[64:128, 128 + 32 + m + rep] = W1[m, 0:64]    # rgbB
            w1sb[64:96, 256 + m + rep] = W1[m, 64:96]         # evA
            w1sb[96:128, 32 + m + rep] = W1[m, 64:96]         # evB

    # mm2 block-diag lhsT per tap: [K=64 (hA|hB), M=128 (cA|cB)], 2 copies
    W2r = W2.reshape(C, 9, MID)
    w2sb = np.zeros((128, 9 * 128), np.float32)
    for ij in range(9):
        blk = np.zeros((64, 128), np.float32)
        blk[0:32, 0:64] = W2r[:, ij, :].T
        blk[32:64, 64:128] = W2r[:, ij, :].T
        w2sb[0:64, 128 * ij:128 * ij + 128] = blk
        w2sb[64:128, 128 * ij:128 * ij + 128] = blk

    bisb = np.zeros((128, 10), np.float32)
    bisb[:, 0] = np.tile(b1, 4)
    b2r = b2.reshape(C, 9)
    for ij in range(9):
        bisb[:, 1 + ij] = np.concatenate([b2r[:, ij], b2r[:, ij]])
    return w1sb.astype(BF), w2sb.astype(BF), bisb


def _shard_inputs(rgb_feature, event_feature, W1, b1, W2, b2):
    rgbp = np.pad(rgb_feature, ((0, 0), (0, 0), (1, 1), (1, 1)), mode="reflect")
    # two bf16 copies of the padded slab: pixel col c at element c+2 (even
    # view, serves dj=0) and at element c+1 (odd view, serves dj=+-1).
    rgbe = np.zeros((B, C, H + 2, WE), BF)
    rgbo = np.zeros((B, C, H + 2, WE), BF)
    rgbe[:, :, :, 1:1 + W + 2] = rgbp
    rgbo[:, :, :, 0:W + 2] = rgbp
    evb = event_feature.astype(BF)
    w1sb, w2sb, bisb = _prep_consts(W1, b1, W2, b2)
    in_maps = []
    for k in range(NCORES):
        b, r0 = k // 2, SHARD_H * (k % 2)
        in_maps.append({
            "rgbe": np.ascontiguousarray(rgbe[b, :, r0:r0 + SHARD_H + 2, :]),
            "rgbo": np.ascontiguousarray(rgbo[b, :, r0:r0 + SHARD_H + 2, :]),
            "ev": np.ascontiguousarray(evb[b, :, r0:r0 + SHARD_H, :]),
            "w1": w1sb, "w2": w2sb, "bi": bisb,
        })
    return in_maps


def _run(inputs, trace=False, **trace_kwargs):
    if "nc" not in _cache:
        _cache["nc"] = _build()
    nc = _cache["nc"]
    in_maps = _shard_inputs(
        inputs["rgb_feature"].astype(np.float32),
        inputs["event_feature"].astype(np.float32),
        inputs["W1"].astype(np.float32), inputs["b1"].astype(np.float32),
        inputs["W2"].astype(np.float32), inputs["b2"].astype(np.float32))
    res = run_bass_kernel_spmd(nc, in_maps, list(range(NCORES)),
                               trace=trace, **trace_kwargs)
    full = np.empty((B, C, H, W), np.float32)
    for k in range(NCORES):
        b, r0 = k // 2, SHARD_H * (k % 2)
        full[b, :, r0:r0 + SHARD_H, :] = res.results[k]["out"].astype(np.float32)
    return full, res


def kernel(**inputs):
    full, _ = _run(inputs, trace=False)
    return full


# revision 9
# speedup vs baseline: 1.0949x; 1.0949x over previous
"""Event-RGB dynamic fusion module on 8 trn2 NeuronCores (v2).

Per-pixel dynamic 3x3 depthwise kernels predicted from concat(rgb, event)
via two 1x1 convs + relu, applied to reflect-padded rgb.

Sharding: 8 shards = (batch b in 0..3) x (H half in {0,1}); each core gets
reflect-padded rgb slabs (two bf16 copies at element offsets 0/1 so every
3x3-shift view stays 4-byte aligned for DVE 2x mode), a bf16 event slab,
and replicated pre-laid-out bf16 weights. Fully data-parallel, no
collectives.

v2 changes vs v1:
- mm1/mm2 merge the two 64-row pixel halves into single matmuls via
  block-structured weights (A-half at psum partitions 0-63, B at 64-127
  for mm2; interleaved 32-col groups for mm1), halving PE stream time.
- mm2 lhsT is block-diagonal [K=64 (hA|hB), M=128 (cA|cB)], one tap per
  matmul pair (2x N=512), dk tile [128 = cA|cB, 1024 px].
- apply: 8 of 9 taps are copied PSUM->SBUF bf16 (bias fused) by ACT (7)
  and DVE (1), pair-multiplied against 4D overlapping patch views as
  2x-mode TTs on DVE/GPSIMD; center tap S stays a direct 1x STT from
  PSUM. Add tree operates on [128,2048] pair tiles.
- output is written bf16 and widened to fp32 on the host.
"""

import os
from contextlib import ExitStack

import ml_dtypes
import numpy as np

import bass_rust
import concourse.bass as bass
import concourse.bacc as bacc
import concourse.mybir as mybir
import concourse.tile as tile
from concourse.bass_utils import run_bass_kernel_spmd

B, C, H, W = 4, 64, 256, 256
CEV, KK, MID = 32, 3, 32
NCORES = 8
SHARD_H = 128          # rows per core
HALF = 64              # rows per half (partition-packing of pixel halves)
RBLK = 16              # rows per half per block
NBLK = HALF // RBLK    # 4
WE = 260               # padded row length (even, so shifted views stay aligned)
SUBR = 4               # rows per half per sub-slice (=1024 px per half)
NSUB = RBLK // SUBR    # 4
F32 = mybir.dt.float32
BF16 = mybir.dt.bfloat16
AOP = mybir.AluOpType
RELU = mybir.ActivationFunctionType.Relu
IDENT = mybir.ActivationFunctionType.Identity
BF = ml_dtypes.bfloat16

# tap index ij = 3*(di+1) + (dj+1)
S_TAP = 4                       # (0,0) center: direct STT from PSUM on DVE
# pairs: (name, [ij_a, ij_b], slab, tap_stride_elems, base_row_off, base_col)
# slab "e" = even (dj=0, col base 2), "o" = odd (dj=-1 -> 0, dj=+1 -> 2)
# O2 first: its product runs on GPSIMD, which is slow — start it earliest.
PAIRS = [
    ("O2", [3, 5], "o", 2,       0, 0),   # (0,-1) & (0,+1)
    ("O3", [6, 8], "o", 2,      +1, 0),   # (+1,-1) & (+1,+1)
    ("E",  [1, 7], "e", 2 * WE, -1, 2),   # (-1,0) & (+1,0): row-stride pair
    ("O1", [0, 2], "o", 2,      -1, 0),   # (-1,-1) & (-1,+1): col-stride pair
]
# engine for each pair product: "D"=DVE tensor_tensor, "G"=GPSIMD
PROD_ENG = {"E": "D", "O1": "D", "O2": "G", "O3": "D"}
# engine for each of the 8 tap copies (pair name, half): "A"=ACT, "V"=DVE
COPY_ENG = {("E", 0): "A", ("E", 1): "A", ("O1", 0): "A", ("O1", 1): "A",
            ("O2", 0): "A", ("O2", 1): "A", ("O3", 0): "A", ("O3", 1): "A"}
# engines for adds: T=E+O1, U=O2+O3 (2048 each), V=T+U, Wf=V0+V1, out=Wf+S
ADD_ENG = {"T": "D", "U": "G", "V": "D", "Wf": "D", "out": "D"}

_cache = {}


def _pair_view(flat_ap, pstride, tap_stride, base):
    """Overlapping [128, 2, 4, 256] view of a flat [128, N] slab tile."""
    v = flat_ap.copy()
    v.ap = bass_rust.VecI64Pair(
        [[pstride, 128], [tap_stride, 2], [WE, SUBR], [1, W]])
    v.offset = flat_ap.offset + base
    return v


def _build():
    nc = bacc.Bacc("TRN2", target_bir_lowering=False, debug=False)
    rgbe = nc.dram_tensor("rgbe", [C, SHARD_H + 2, WE], BF16, kind="ExternalInput").ap()
    rgbo = nc.dram_tensor("rgbo", [C, SHARD_H + 2, WE], BF16, kind="ExternalInput").ap()
    ev = nc.dram_tensor("ev", [CEV, SHARD_H, W], BF16, kind="ExternalInput").ap()
    w1 = nc.dram_tensor("w1", [128, 384], BF16, kind="ExternalInput").ap()
    w2 = nc.dram_tensor("w2", [128, 9 * 128], BF16, kind="ExternalInput").ap()
    bi = nc.dram_tensor("bi", [128, 10], F32, kind="ExternalInput").ap()
    out = nc.dram_tensor("out", [C, SHARD_H, W], BF16, kind="ExternalOutput").ap()

    with tile.TileContext(nc) as tc, ExitStack() as ctx:
        _kernel(ctx, tc, rgbe, rgbo, ev, w1, w2, bi, out)
    nc.compile()
    return nc


def _kernel(ctx, tc, rgbe, rgbo, ev, w1, w2, bi, out):
    nc = tc.nc
    consts = ctx.enter_context(tc.tile_pool(name="consts", bufs=1))
    rgb_p = ctx.enter_context(tc.tile_pool(name="rgb", bufs=2))
    ev_p = ctx.enter_context(tc.tile_pool(name="evp", bufs=2))
    h4_p = ctx.enter_context(tc.tile_pool(name="h4", bufs=2))
    dkb_p = ctx.enter_context(tc.tile_pool(name="dkb", bufs=5))
    sprod_p = ctx.enter_context(tc.tile_pool(name="sprod", bufs=3))
    prod_p = ctx.enter_context(tc.tile_pool(name="prod", bufs=8))
    acc_p = ctx.enter_context(tc.tile_pool(name="acc", bufs=6))
    outt_p = ctx.enter_context(tc.tile_pool(name="outt", bufs=4))
    ph_p = ctx.enter_context(tc.tile_pool(name="psum_h", bufs=1, space="PSUM"))
    pdk_p = ctx.enter_context(tc.tile_pool(name="psum_dk", bufs=3, space="PSUM"))

    w1t = consts.tile([128, 384], BF16)
    nc.sync.dma_start(w1t[:], w1[:])
    w2t = consts.tile([128, 9 * 128], BF16)
    nc.sync.dma_start(w2t[:], w2[:])
    bt = consts.tile([128, 10], F32)
    nc.sync.dma_start(bt[:], bi[:])

    pend = [None]   # software-pipelined tail state (previous sub-slice)

    for t in range(NBLK):
        rge = rgb_p.tile([128, (RBLK + 2) * WE], BF16, tag="rge")
        nc.sync.dma_start(rge[0:64, :], rgbe[:, t * RBLK:t * RBLK + RBLK + 2, :])
        nc.sync.dma_start(
            rge[64:128, :], rgbe[:, HALF + t * RBLK:HALF + t * RBLK + RBLK + 2, :])
        rgo = rgb_p.tile([128, (RBLK + 2) * WE], BF16, tag="rgo")
        nc.sync.dma_start(rgo[0:64, :], rgbo[:, t * RBLK:t * RBLK + RBLK + 2, :])
        nc.sync.dma_start(
            rgo[64:128, :], rgbo[:, HALF + t * RBLK:HALF + t * RBLK + RBLK + 2, :])
        evt = ev_p.tile([128, RBLK * W], BF16)
        nc.sync.dma_start(evt[64:96, :], ev[:, t * RBLK:t * RBLK + RBLK, :])
        nc.sync.dma_start(
            evt[96:128, :], ev[:, HALF + t * RBLK:HALF + t * RBLK + RBLK, :])

        rgev = rge[:].rearrange("p (r w) -> p r w", w=WE)      # [128, 18, 260]
        rgov = rgo[:].rearrange("p (r w) -> p r w", w=WE)
        evv = evt[:].rearrange("p (r w) -> p r w", w=W)        # [128, 16, 256]
        pstride_e = rge[:].ap[0][0]
        pstride_o = rgo[:].ap[0][0]

        def tadd(key, shape, a, b, pool=acc_p):
            r = pool.tile(shape, BF16, tag="acc", name=f"acc{key}")
            eng = nc.gpsimd if ADD_ENG[key] == "G" else nc.vector
            eng.tensor_tensor(r[:], a, b, op=AOP.add)
            return r

        def flush_tail(st):
            # tail adds of the previous sub-slice, emitted late so the DVE
            # queue never stalls waiting on the GPSIMD "U" partial sum.
            tV = tadd("V", [128, 2048], st["T"][:], st["U"][:])
            tW = tadd("Wf", [128, 1024], tV[:, 0:1024], tV[:, 1024:2048])
            ot = tadd("out", [128, 1024], tW[:], st["S"][:], pool=outt_p)
            otv = ot[:].rearrange("p (r w) -> p r w", w=W)
            ra = st["ra"]
            nc.sync.dma_start(out[:, ra:ra + SUBR, :], otv[0:64, :, :])
            nc.sync.dma_start(out[:, HALF + ra:HALF + ra + SUBR, :],
                              otv[64:128, :, :])

        for s in range(NSUB):
            r0 = SUBR * s
            # ---- mm1: h4[128, 1024] = relu(b1 + W1 @ concat(rgb, ev)),
            # A-half at psum partitions {0-31, 64-95}, B at {32-63, 96-127}.
            # NB: two K=64 matmuls at (0,0)+(64,0) in ONE accumulation group
            # hang the PE on hw; split into two baseline-style K64+K32
            # groups, the second opening with start=False so the PSUM bank
            # keeps accumulating (skip_group_check for the simulator).
            # Emission is interleaved across the two banks (nh) so adjacent
            # PE streams hit different banks and overlap.
            ph = ph_p.tile([128, 2 * 512], F32)
            dsts = [ph[:, 0:512], ph[:, 512:1024]]
            prs = [r0, r0 + 2]
            for lhs, rv, rows_off, st, sp_, skip in (
                    (w1t[0:64, 0:128], "rg", 1, True, False, False),
                    (w1t[64:96, 256:384], "ev", 0, False, True, False),
                    (w1t[64:128, 128:256], "rg", 1, False, False, True),
                    (w1t[96:128, 0:128], "ev", 0, False, True, True)):
                for nh in range(2):
                    pr = prs[nh] + rows_off
                    if rv == "rg":
                        pp = 0 if not skip else 64
                        rhs = rgev[pp:pp + 64, pr:pr + 2, 2:258]
                        pos = (pp, 0)
                    else:
                        pp = 64 if not skip else 96
                        rhs = evv[pp:pp + 32, pr:pr + 2, :]
                        pos = (pp, 0)
                    nc.tensor.matmul(dsts[nh], lhs, rhs, start=st, stop=sp_,
                                     tile_position=pos,
                                     skip_group_check=skip)
            h4 = h4_p.tile([128, 1024], BF16)
            nc.scalar.activation(h4[:], ph[:], RELU, bias=bt[:, 0:1], scale=1.0)

            # ---- mm2 + apply: 4 pairs + center tap ----
            # matmuls of a pair are interleaved (a-nh0, b-nh0, a-nh1, b-nh1):
            # adjacent streams use different PE tiles + PSUM banks -> overlap.
            prods = {}
            cp = 0
            for name, ijs, slab, tstride, drow, bcol in PAIRS:
                dks = []
                for ij in ijs:
                    dks.append(pdk_p.tile([128, 1024], F32,
                                          name=f"dk{ij}", tag="dk"))
                for nh in range(2):
                    for k, ij in enumerate(ijs):
                        cpk = (cp + k) % 2
                        nc.tensor.matmul(
                            dks[k][:, 512 * nh:512 * nh + 512],
                            w2t[64 * cpk:64 * cpk + 64,
                                128 * ij:128 * ij + 128],
                            h4[64 * cpk:64 * cpk + 64,
                               512 * nh:512 * nh + 512],
                            start=True, stop=True, tile_position=(64 * cpk, 0))
                cp ^= 1
                dkb = dkb_p.tile([128, 2048], BF16)
                for hfi, ij in enumerate(ijs):
                    half = dkb[:, 1024 * hfi:1024 * hfi + 1024]
                    if COPY_ENG[(name, hfi)] == "A":
                        nc.scalar.activation(half, dks[hfi][:], IDENT,
                                             bias=bt[:, 1 + ij:2 + ij], scale=1.0)
                    else:
                        nc.vector.tensor_scalar_add(half, dks[hfi][:],
                                                    bt[:, 1 + ij:2 + ij])
                flat = rge[:] if slab == "e" else rgo[:]
                pstr = pstride_e if slab == "e" else pstride_o
                base = (r0 + 1 + drow) * WE + bcol
                patch = _pair_view(flat, pstr, tstride, base)
                prod = prod_p.tile([128, 2048], BF16)
                dkbv = dkb[:].rearrange("p (t r w) -> p t r w", t=2, w=W)
                prodv = prod[:].rearrange("p (t r w) -> p t r w", t=2, w=W)
                eng = nc.gpsimd if PROD_ENG[name] == "G" else nc.vector
                eng.tensor_tensor(prodv[:], dkbv[:], patch, op=AOP.mult)
                prods[name] = prod
                if name == "O3":
                    tU = tadd("U", [128, 2048], prods["O2"][:], prods["O3"][:])

            # center tap: direct STT from PSUM
            dkS = pdk_p.tile([128, 1024], F32, name="dkS", tag="dk")
            for nh in range(2):
                nc.tensor.matmul(
                    dkS[:, 512 * nh:512 * nh + 512],
                    w2t[64 * cp:64 * cp + 64, 128 * S_TAP:128 * S_TAP + 128],
                    h4[64 * cp:64 * cp + 64, 512 * nh:512 * nh + 512],
                    start=True, stop=True, tile_position=(64 * cp, 0))
            sp = sprod_p.tile([128, 1024], BF16)
            spv = sp[:].rearrange("p (r w) -> p r w", w=W)
            patchS = rgev[:, r0 + 1:r0 + 5, 2:258]
            nc.vector.scalar_tensor_tensor(spv[:], dkS[:],
                                           bt[:, 1 + S_TAP:2 + S_TAP],
                                           patchS[:], op0=AOP.add, op1=AOP.mult)
            tT = tadd("T", [128, 2048], prods["E"][:], prods["O1"][:])

            if pend[0] is not None:
                flush_tail(pend[0])
            pend[0] = {"T": tT, "U": tU, "S": sp, "ra": t * RBLK + r0,
                       "flush": flush_tail}
    pend[0]["flush"](pend[0])


def _prep_consts(W1, b1, W2, b2):
    # mm1 lhsT blocks: psum col m -> {0-31: mA, 32-63: mB, 64-95: mA, 96-127: mB}
    w1sb = np.zeros((128, 384), np.float32)
    for m in range(32):
        for rep in (0, 64):
            w1sb[0:64, m + rep] = W1[m, 0:64]                 # rgbA
            w1sb[64:128, 128 + 32 + m + rep] = W1[m, 0:64]    # rgbB
            w1sb[64:96, 256 + m + rep] = W1[m, 64:96]         # evA
            w1sb[96:128, 32 + m + rep] = W1[m, 64:96]         # evB

    # mm2 block-diag lhsT per tap: [K=64 (hA|hB), M=128 (cA|cB)], 2 copies
    W2r = W2.reshape(C, 9, MID)
    w2sb = np.zeros((128, 9 * 128), np.float32)
    for ij in range(9):
        blk = np.zeros((64, 128), np.float32)
        blk[0:32, 0:64] = W2r[:, ij, :].T
        blk[32:64, 64:128] = W2r[:, ij, :].T
        w2sb[0:64, 128 * ij:128 * ij + 128] = blk
        w2sb[64:128, 128 * ij:128 * ij + 128] = blk

    bisb = np.zeros((128, 10), np.float32)
    bisb[:, 0] = np.tile(b1, 4)
    b2r = b2.reshape(C, 9)
    for ij in range(9):
        bisb[:, 1 + ij] = np.concatenate([b2r[:, ij], b2r[:, ij]])
    return w1sb.astype(BF), w2sb.astype(BF), bisb


def _shard_inputs(rgb_feature, event_feature, W1, b1, W2, b2):
    rgbp = np.pad(rgb_feature, ((0, 0), (0, 0), (1, 1), (1, 1)), mode="reflect")
    # two bf16 copies of the padded slab: pixel col c at element c+2 (even
    # view, serves dj=0) and at element c+1 (odd view, serves dj=+-1).
    rgbe = np.zeros((B, C, H + 2, WE), BF)
    rgbo = np.zeros((B, C, H + 2, WE), BF)
    rgbe[:, :, :, 1:1 + W + 2] = rgbp
    rgbo[:, :, :, 0:W + 2] = rgbp
    evb = event_feature.astype(BF)
    w1sb, w2sb, bisb = _prep_consts(W1, b1, W2, b2)
    in_maps = []
    for k in range(NCORES):
        b, r0 = k // 2, SHARD_H * (k % 2)
        in_maps.append({
            "rgbe": np.ascontiguousarray(rgbe[b, :, r0:r0 + SHARD_H + 2, :]),
            "rgbo": np.ascontiguousarray(rgbo[b, :, r0:r0 + SHARD_H + 2, :]),
            "ev": np.ascontiguousarray(evb[b, :, r0:r0 + SHARD_H, :]),
            "w1": w1sb, "w2": w2sb, "bi": bisb,
        })
    return in_maps


def _run(inputs, trace=False, **trace_kwargs):
    if "nc" not in _cache:
        _cache["nc"] = _build()
    nc = _cache["nc"]
    in_maps = _shard_inputs(
        inputs["rgb_feature"].astype(np.float32),
        inputs["event_feature"].astype(np.float32),
        inputs["W1"].astype(np.float32), inputs["b1"].astype(np.float32),
        inputs["W2"].astype(np.float32), inputs["b2"].astype(np.float32))
    res = run_bass_kernel_spmd(nc, in_maps, list(range(NCORES)),
                               trace=trace, **trace_kwargs)
    full = np.empty((B, C, H, W), np.float32)
    for k in range(NCORES):
        b, r0 = k // 2, SHARD_H * (k % 2)
        full[b, :, r0:r0 + SHARD_H, :] = res.results[k]["out"].astype(np.float32)
    return full, res


def kernel(**inputs):
    full, _ = _run(inputs, trace=False)
    return full
